# revision 32
# baseline (speedup 1.0000x reference)
"""GAT (3-layer, 4-head) message-passing network on 8 Trainium2 NeuronCores.

Strategy (graph/data parallel, per sharding hint):
  - Nodes partitioned contiguously across 8 cores (6250 dst nodes/core);
    edges partitioned by destination node; GAT weights replicated.
  - Per layer, each core computes g = act @ W for its own nodes plus the
    attention logits al_src/al_dst (via the folded weight W @ [a_src|a_dst]),
    then an AllGather replicates the per-node table [al_src(4) | g(256)]
    so every core can gather arbitrary source rows.
  - Edge pass: edges are grouped by 128-dst-node groups, padded to a uniform
    number of 128-edge tiles. Per tile, one indirect DMA gathers the
    [als|g] rows for the 128 source nodes, attention p = exp(leaky_relu(
    als_src + al_dst)) is computed on ACT/DVE, messages are scaled by p,
    and a one-hot selection matrix S (built by iota-compare against the
    dst offsets) turns the segment-sum into a PE matmul accumulated in
    PSUM: psum[d, :] += S.T @ [p | p*g].  Softmax max-subtraction is skipped
    (logits are provably in [-1.3, 5.4] for this model, exp cannot overflow;
    pad slots gather a -1e38 sentinel row so exp underflows to exactly 0).
  - Epilogue per group: divide by the accumulated denominators, add bias,
    ELU (decomposed as relu(x)-1+exp(min(x,0)) to keep ACT on one table),
    transpose to feature-major via PE transposes, and immediately run the
    next layer's W-matmul for this group so the next AllGather chunk can
    fire while the edge pass continues (compute/collective overlap).
  - Readout: poolvec = act3 @ lin_w per node, segment-sum over (sorted)
    graph ids with the same one-hot-matmul trick, AllReduce over the 8
    cores, + lin_b.
"""

import math
import sys
from contextlib import ExitStack
from dataclasses import dataclass

import numpy as np

sys.path.insert(0, "/opt/trn_rl_repo")

import concourse.bacc as bacc
import concourse.bass as bass
import concourse.tile as tile
from concourse import mybir
from concourse.bass import IndirectOffsetOnAxis
from concourse.bass_utils import run_bass_kernel_spmd

F32 = mybir.dt.float32
I32 = mybir.dt.int32
AF = mybir.ActivationFunctionType
OP = mybir.AluOpType
P = 128
SENTINEL = -1.0e38


def expand_mid(ap: bass.AP, n: int) -> bass.AP:
    """[P, F] -> [P, n, F] with a stride-0 middle dim."""
    pat = ap.ap
    assert len(pat) == 2
    return bass.AP(ap.tensor, ap.offset, [pat[0], [0, n], pat[1]])


@dataclass
class Cfg:
    N: int = 50000
    E: int = 400000
    IN_DIM: int = 128
    HID: int = 64
    HEADS: int = 4
    NUM_GRAPHS: int = 256
    NCORES: int = 8
    NEG: float = 0.2
    CHUNK_GROUPS: int = 7  # groups per AllGather chunk

    @property
    def WIDTH(self):
        return self.HID * self.HEADS

    @property
    def D(self):
        return self.N // self.NCORES

    @property
    def NG(self):
        return (self.D + P - 1) // P

    @property
    def TCOLS(self):
        return 4 + self.WIDTH


FULL = Cfg()


# --------------------------------------------------------------------------
# Host-side sharding / index preparation (graph structure only — no FLOPs
# on model data happen on the host).
# --------------------------------------------------------------------------

def host_prep(cfg: Cfg, inputs: dict):
    N, E, D, NG, NC = cfg.N, cfg.E, cfg.D, cfg.NG, cfg.NCORES
    x = np.asarray(inputs["x"], dtype=np.float32)
    ei = np.asarray(inputs["edge_index"]).astype(np.int64)
    batch = np.asarray(inputs["batch"]).astype(np.int64)

    loops = np.arange(N, dtype=np.int64)
    src = np.concatenate([ei[0], loops])
    dst = np.concatenate([ei[1], loops])
    order = np.argsort(dst, kind="stable")
    src, dst = src[order], dst[order]

    # The gather table is laid out chunk-major ([chunk][core][row]) so each
    # chunked AllGather writes one contiguous row range. Map global node id
    # -> table row.
    dch = cfg.CHUNK_GROUPS * P
    n_all = np.arange(N, dtype=np.int64)
    c_all, i_all = n_all // D, n_all % D
    k_all = i_all // dch
    csize = np.minimum(dch, D - k_all * dch)
    row_of = NC * k_all * dch + c_all * csize + (i_all - k_all * dch)
    src_row = row_of[src]

    core_of = dst // D
    # group counts per (core, group) to find the uniform tile capacity
    grp = (dst % D) // P
    counts = np.zeros((NC, NG), dtype=np.int64)
    np.add.at(counts, (core_of, grp), 1)
    T = int(math.ceil(counts.max() / P))  # tiles per group (uniform)

    in_maps = []
    for c in range(NC):
        m = core_of == c
        s_c, d_c = src_row[m], dst[m] - c * D
        g_c = d_c // P
        # stable order within core is already by dst; slot edges per group
        srcidx = np.zeros((NG, P, T), dtype=np.int32)    # pad -> row 0 (p==0)
        dstloc = np.full((NG, P, T), D, dtype=np.int32)  # pad -> sentinel row
        dstoff = np.zeros((NG, P, T), dtype=np.float32)
        for g in range(NG):
            gm = g_c == g
            cnt = int(gm.sum())
            sl = np.empty(T * P, dtype=np.int32)
            dl = np.empty(T * P, dtype=np.int32)
            do = np.zeros(T * P, dtype=np.float32)
            sl[:cnt] = s_c[gm]
            dl[:cnt] = d_c[gm]
            do[:cnt] = (d_c[gm] - g * P).astype(np.float32)
            sl[cnt:] = 0
            dl[cnt:] = D
            # reshape to [T, P] then transpose -> [P, T] slot layout
            srcidx[g] = sl.reshape(T, P).T
            dstloc[g] = dl.reshape(T, P).T
            dstoff[g] = do.reshape(T, P).T

        batch_pad = np.full((NG * P, 1), cfg.NUM_GRAPHS + 10, dtype=np.float32)
        batch_pad[:D, 0] = batch[c * D : (c + 1) * D].astype(np.float32)

        consts = np.zeros((P, 3 * P), dtype=np.float32)
        consts[:, 0:P] = np.eye(P, dtype=np.float32)
        consts[:, P : 2 * P] = np.arange(P, dtype=np.float32)[None, :]
        consts[:, 2 * P : 3 * P] = P + np.arange(P, dtype=np.float32)[None, :]

        im = {
            "consts": consts,
            "xT": np.ascontiguousarray(x[c * D : (c + 1) * D].T),  # [IN_DIM, D]
            "srcidx": np.ascontiguousarray(srcidx.reshape(NG * P, T)),
            "dstloc": np.ascontiguousarray(dstloc.reshape(NG * P, T)),
            "dstoff": np.ascontiguousarray(dstoff.reshape(NG * P, T)),
            "batchpad": batch_pad,
            "linw": np.asarray(inputs["lin_w"], dtype=np.float32).reshape(cfg.WIDTH, 1),
            "linb": np.asarray(inputs["lin_b"], dtype=np.float32).reshape(1, 1),
        }
        for l in range(3):
            W = np.asarray(inputs[f"W{l}"], dtype=np.float32)
            a_s = np.asarray(inputs[f"a_src{l}"], dtype=np.float32)
            a_d = np.asarray(inputs[f"a_dst{l}"], dtype=np.float32)
            b = np.asarray(inputs[f"b{l}"], dtype=np.float32)
            im[f"W{l}"] = np.ascontiguousarray(W)               # [in, 256]
            im[f"WT{l}"] = np.ascontiguousarray(W.T)            # [256, in]
            # a_s: [HEADS, HID]; Acat[c_out, j]: c_out = h*HID + cc
            A = np.zeros((cfg.WIDTH, 8), dtype=np.float32)
            for h in range(cfg.HEADS):
                A[h * cfg.HID : (h + 1) * cfg.HID, h] = a_s[h]
                A[h * cfg.HID : (h + 1) * cfg.HID, 4 + h] = a_d[h]
            im[f"Acat{l}"] = A
            im[f"bias{l}"] = b.reshape(1, cfg.WIDTH)
        in_maps.append(im)

    return in_maps, T


# --------------------------------------------------------------------------
# Device program
# --------------------------------------------------------------------------

def build_nc(cfg: Cfg, T: int):
    N, D, NG, NC = cfg.N, cfg.D, cfg.NG, cfg.NCORES
    W_, TC, H, HID = cfg.WIDTH, cfg.TCOLS, cfg.HEADS, cfg.HID
    IN = cfg.IN_DIM
    RG = [list(range(NC))]
    n_in_halves = [IN // P, W_ // P, W_ // P]  # per layer

    nc = bacc.Bacc("TRN2", target_bir_lowering=False, debug=False,
                   num_devices=NC, enable_asserts=False)

    # ---- I/O ----
    t_xT = nc.dram_tensor("xT", [IN, D], F32, kind="ExternalInput")
    t_srcidx = nc.dram_tensor("srcidx", [NG * P, T], I32, kind="ExternalInput")
    t_dstloc = nc.dram_tensor("dstloc", [NG * P, T], I32, kind="ExternalInput")
    t_dstoff = nc.dram_tensor("dstoff", [NG * P, T], F32, kind="ExternalInput")
    t_batch = nc.dram_tensor("batchpad", [NG * P, 1], F32, kind="ExternalInput")
    t_W, t_WT, t_A, t_b = {}, {}, {}, {}
    for l in range(3):
        nin = [IN, W_, W_][l]
        t_W[l] = nc.dram_tensor(f"W{l}", [nin, W_], F32, kind="ExternalInput")
        t_WT[l] = nc.dram_tensor(f"WT{l}", [W_, nin], F32, kind="ExternalInput")
        t_A[l] = nc.dram_tensor(f"Acat{l}", [W_, 8], F32, kind="ExternalInput")
        t_b[l] = nc.dram_tensor(f"bias{l}", [1, W_], F32, kind="ExternalInput")
    t_linw = nc.dram_tensor("linw", [W_, 1], F32, kind="ExternalInput")
    t_linb = nc.dram_tensor("linb", [1, 1], F32, kind="ExternalInput")
    t_consts = nc.dram_tensor("consts", [P, 3 * P], F32, kind="ExternalInput")
    t_out = nc.dram_tensor("out", [cfg.NUM_GRAPHS, 1], F32, kind="ExternalOutput")

    with tile.TileContext(nc) as tc, ExitStack() as ctx:
        const = ctx.enter_context(tc.tile_pool(name="const", bufs=1))
        work = ctx.enter_context(tc.tile_pool(name="work", bufs=3))
        gpool = ctx.enter_context(tc.tile_pool(name="gath", bufs=3))
        psE = ctx.enter_context(tc.tile_pool(name="psE", bufs=2, space="PSUM"))
        psA = ctx.enter_context(tc.tile_pool(name="psA", bufs=2, space="PSUM"))
        psP = ctx.enter_context(tc.tile_pool(name="psP", bufs=1, space="PSUM"))
        dram = ctx.enter_context(tc.tile_pool(name="dram", bufs=1, space="DRAM"))

        # ---- DRAM scratch ----
        # NOTE: Local (not Shared) so multiple chunked AllGathers may write
        # disjoint row ranges — Shared DRAM enforces a single writer inst.
        gtab = [dram.tile([N, TC], F32, tag=f"gtab{l}", name=f"gtab{l}")
                for l in range(3)]
        gin = [dram.tile([D, TC], F32, tag=f"gin{l}", name=f"gin{l}")
               for l in range(3)]
        aldtab = [dram.tile([D + 1, 4], F32, tag=f"ald{l}", name=f"ald{l}")
                  for l in range(3)]
        ar_in = dram.tile([cfg.NUM_GRAPHS, 1], F32, tag="arin")
        ar_out = dram.tile([cfg.NUM_GRAPHS, 1], F32, tag="arout", addr_space="Shared")

        # ---- constants (identity | iota row | iota row + 128, from host) ----
        cns = const.tile([P, 3 * P], F32)
        nc.sync.dma_start(out=cns[:], in_=t_consts[:, :])
        ident = cns[:, 0:P]
        iotaA = cns[:, P : 2 * P]
        iotaB = cns[:, 2 * P : 3 * P]
        ones_row = const.tile([1, P], F32)
        nc.vector.memset(ones_row[:], 1.0)

        # sentinel row (only in the local al_dst table: pad edge slots gather
        # al_dst = -1e38 so p = exp(lrelu(als - 1e38)) == 0 exactly)
        sent = const.tile([1, 4], F32)
        nc.vector.memset(sent[:], SENTINEL)
        for l in range(3):
            nc.sync.dma_start(out=aldtab[l][D : D + 1, :], in_=sent[:1, :4])

        # weights to SBUF
        Wsb = {}
        for l in range(3):
            nh = n_in_halves[l]
            Wsb[l] = []
            for hf in range(nh):
                wt = const.tile([P, W_], F32, tag=f"W{l}_{hf}")
                nc.sync.dma_start(out=wt[:], in_=t_W[l][hf * P : (hf + 1) * P, :])
                Wsb[l].append(wt)

        # WA[l] = W @ Acat  (on-device, tiny): WA[ci, j] = sum_co WT[co, ci] A[co, j]
        WAsb = {}
        for l in range(3):
            nh = n_in_halves[l]
            acat_sb = work.tile([P, 8], F32, tag="acat")
            acat_sb2 = work.tile([P, 8], F32, tag="acat2")
            nc.sync.dma_start(out=acat_sb[:], in_=t_A[l][0:P, :])
            nc.sync.dma_start(out=acat_sb2[:], in_=t_A[l][P : 2 * P, :])
            WAsb[l] = []
            for hf in range(nh):
                wtc = work.tile([P, P], F32, tag="wtc")
                wtc2 = work.tile([P, P], F32, tag="wtc2")
                nc.sync.dma_start(out=wtc[:, :],
                                  in_=t_WT[l][0:P, hf * P : (hf + 1) * P])
                nc.sync.dma_start(out=wtc2[:, :],
                                  in_=t_WT[l][P : 2 * P, hf * P : (hf + 1) * P])
                waps = psA.tile([P, 8], F32, tag="gpsum")
                nc.tensor.matmul(waps[:], lhsT=wtc[:], rhs=acat_sb[:],
                                 start=True, stop=False)
                nc.tensor.matmul(waps[:], lhsT=wtc2[:], rhs=acat_sb2[:],
                                 start=False, stop=True)
                wa = const.tile([P, 8], F32, tag=f"WA{l}_{hf}")
                nc.vector.tensor_copy(wa[:], waps[:])
                WAsb[l].append(wa)

        # biases replicated across partitions via ones-column matmul
        brep = {}
        for l in range(3):
            bt = work.tile([1, W_], F32, tag="btmp")
            nc.sync.dma_start(out=bt[:1, :], in_=t_b[l][:1, :])
            bps = psA.tile([P, W_], F32, tag="tpose")
            nc.tensor.matmul(bps[:], lhsT=ones_row[:1, :], rhs=bt[:1, :],
                             start=True, stop=True)
            br = const.tile([P, W_], F32, tag=f"brep{l}")
            nc.vector.tensor_copy(br[:], bps[:])
            brep[l] = br
        lwsb = []
        for hf in range(W_ // P):
            lw = const.tile([P, 1], F32, tag=f"lw{hf}")
            nc.sync.dma_start(out=lw[:], in_=t_linw[hf * P : (hf + 1) * P, :])
            lwsb.append(lw)
        lbt = work.tile([1, 1], F32, tag="btmp2")
        nc.sync.dma_start(out=lbt[:1, :], in_=t_linb[:1, :])
        lbps = psA.tile([P, 1], F32, tag="gpsum")
        nc.tensor.matmul(lbps[:], lhsT=ones_row[:1, :], rhs=lbt[:1, :],
                         start=True, stop=True)
        linb_bc = const.tile([P, 1], F32)
        nc.vector.tensor_copy(linb_bc[:], lbps[:])

        xT_sb = const.tile([IN, NG * P], F32)
        if NG * P > D:
            nc.vector.memset(xT_sb[:, D:], 0.0)
        nc.sync.dma_start(out=xT_sb[:, :D], in_=t_xT[:, :])

        n_chunks = (NG + cfg.CHUNK_GROUPS - 1) // cfg.CHUNK_GROUPS

        def fire_ag_chunks(l, g):
            """After finishing group g of phase-1 for layer l, fire any AG chunk."""
            k = g // cfg.CHUNK_GROUPS
            if g == min((k + 1) * cfg.CHUNK_GROUPS, NG) - 1:
                r0 = k * cfg.CHUNK_GROUPS * P
                r1 = min((k + 1) * cfg.CHUNK_GROUPS * P, D)
                # chunk-major table: chunk k occupies contiguous rows
                # [NC*r0, NC*r0 + NC*(r1-r0))
                o0 = NC * r0
                o1 = o0 + NC * (r1 - r0)
                nc.gpsimd.collective_compute(
                    "AllGather", OP.bypass, replica_groups=RG,
                    ins=[gin[l][r0:r1, :].opt()],
                    outs=[gtab[l][o0:o1, :].opt()],
                )

        def phase1_group(l, g, lhs_halves):
            """Compute [al(8) | g(256)] for group g of layer l from the
            feature-major activation halves ([128 c, 128 n] APs each);
            store to gin/aldtab; fire AG chunk when due."""
            ng = min(P, D - g * P)
            nh = n_in_halves[l]
            assert len(lhs_halves) == nh
            gps = psA.tile([P, 8 + W_], F32, tag="gpsum")
            for hf in range(nh):
                nc.tensor.matmul(gps[:, 8:], lhsT=lhs_halves[hf], rhs=Wsb[l][hf][:],
                                 start=(hf == 0), stop=(hf == nh - 1))
            for hf in range(nh):
                nc.tensor.matmul(gps[:, 0:8], lhsT=lhs_halves[hf], rhs=WAsb[l][hf][:],
                                 start=(hf == 0), stop=(hf == nh - 1))
            gfu = work.tile([P, TC], F32, tag="gfu")
            nc.vector.tensor_copy(gfu[:, 0:4], gps[:, 0:4])
            nc.vector.tensor_copy(gfu[:, 4:], gps[:, 8:])
            alds = work.tile([P, 4], F32, tag="aldsb")
            nc.vector.tensor_copy(alds[:], gps[:, 4:8])
            nc.sync.dma_start(out=gin[l][g * P : g * P + ng, :], in_=gfu[:ng, :])
            nc.sync.dma_start(out=aldtab[l][g * P : g * P + ng, :], in_=alds[:ng, :])
            fire_ag_chunks(l, g)

        # ---- phase-1 for layer 0 (from host-provided xT) ----
        for g in range(NG):
            halves = [xT_sb[hf * P : (hf + 1) * P, g * P : g * P + P]
                      for hf in range(n_in_halves[0])]
            phase1_group(0, g, halves)

        # persistent pool psums (layer-2 readout accumulators)
        poolA = psP.tile([P, 1], F32, tag="poolA")
        poolB = psP.tile([P, 1], F32, tag="poolB")

        # ---- layers ----
        for l in range(3):
            for g in range(NG):
                ng = min(P, D - g * P)
                sidx = work.tile([P, T], I32, tag="sidx")
                nc.sync.dma_start(out=sidx[:], in_=t_srcidx[g * P : (g + 1) * P, :])
                dlix = work.tile([P, T], I32, tag="dlix")
                nc.sync.dma_start(out=dlix[:], in_=t_dstloc[g * P : (g + 1) * P, :])
                doff = work.tile([P, T], F32, tag="doff")
                nc.sync.dma_start(out=doff[:], in_=t_dstoff[g * P : (g + 1) * P, :])

                # NB: HW indirect DMA honors ONE index per partition per
                # instruction (multi-column idx batching works only in sim)
                tbl = gpool.tile([P, T * TC], F32, tag="tbl")
                for t in range(T):
                    nc.gpsimd.indirect_dma_start(
                        out=tbl[:, t * TC : (t + 1) * TC], out_offset=None,
                        in_=gtab[l][:, :],
                        in_offset=IndirectOffsetOnAxis(ap=sidx[:, t : t + 1],
                                                       axis=0))
                aldt = work.tile([P, T * 4], F32, tag="aldt")
                for t in range(T):
                    nc.gpsimd.indirect_dma_start(
                        out=aldt[:, t * 4 : (t + 1) * 4], out_offset=None,
                        in_=aldtab[l][:, :],
                        in_offset=IndirectOffsetOnAxis(ap=dlix[:, t : t + 1],
                                                       axis=0))

                tbl3 = tbl[:].rearrange("p (t w) -> p t w", t=T)
                # s = als_src + al_dst ; e = lrelu(s) ; p = exp(e)
                stl = work.tile([P, T * 4], F32, tag="stl")
                nc.vector.tensor_tensor(out=stl[:].rearrange("p (t f) -> p t f", t=T),
                                        in0=tbl3[:, :, 0:4], in1=aldt[:].rearrange(
                                            "p (t f) -> p t f", t=T), op=OP.add)
                etl = work.tile([P, T * 4], F32, tag="etl")
                nc.vector.tensor_scalar_mul(etl[:], stl[:], cfg.NEG)
                nc.vector.tensor_tensor(etl[:], etl[:], stl[:], op=OP.max)
                ptl = work.tile([P, T * 4], F32, tag="ptl")
                nc.scalar.activation(ptl[:], etl[:], AF.Exp)
                ptl3 = ptl[:].rearrange("p (t f) -> p t f", t=T)
                # write p into the als slots of tbl (matmul rhs = [p | p*g])
                nc.vector.tensor_copy(tbl3[:, :, 0:4], ptl3)
                # scale messages in place: tbl[:, t, 4:] *= p (per head)
                for t in range(T):
                    nc.vector.tensor_tensor(
                        out=tbl[:, t * TC + 4 : (t + 1) * TC].rearrange(
                            "p (h c) -> p h c", h=H),
                        in0=tbl[:, t * TC + 4 : (t + 1) * TC].rearrange(
                            "p (h c) -> p h c", h=H),
                        in1=ptl[:, t * 4 : (t + 1) * 4].to_broadcast([P, H, HID]),
                        op=OP.mult)
                # selection matrices for all tiles: S[e, d] = (dstoff[e,t]==d)
                sbig = gpool.tile([P, T * P], F32, tag="sbig")
                nc.vector.tensor_tensor(
                    out=sbig[:].rearrange("p (t d) -> p t d", t=T),
                    in0=doff[:].to_broadcast([P, T, P]),
                    in1=expand_mid(iotaA, T),
                    op=OP.is_equal)

                pse = psE.tile([P, TC], F32, tag="edge")
                for t in range(T):
                    nc.tensor.matmul(pse[:], lhsT=sbig[:, t * P : (t + 1) * P],
                                     rhs=tbl[:, t * TC : (t + 1) * TC],
                                     start=(t == 0), stop=(t == T - 1))

                # ---- epilogue: softmax divide, bias, ELU ----
                rcp = work.tile([P, 4], F32, tag="rcp")
                nc.vector.tensor_scalar_add(rcp[:], pse[:, 0:4], 1e-16)
                nc.vector.reciprocal(rcp[:], rcp[:])
                act = work.tile([P, W_], F32, tag="act")
                nc.vector.tensor_tensor(
                    out=act[:].rearrange("p (h c) -> p h c", h=H),
                    in0=pse[:, 4:].rearrange("p (h c) -> p h c", h=H),
                    in1=rcp[:].to_broadcast([P, H, HID]), op=OP.mult)
                nc.vector.tensor_add(act[:], act[:], brep[l][:])
                mt = work.tile([P, W_], F32, tag="mt")
                nc.vector.tensor_scalar_min(mt[:], act[:], 0.0)
                mt2 = work.tile([P, W_], F32, tag="mt2")
                nc.scalar.activation(mt2[:], mt[:], AF.Exp)
                nc.vector.tensor_scalar(act[:], act[:], 0.0, -1.0,
                                        OP.max, OP.add)
                nc.vector.tensor_add(act[:], act[:], mt2[:])

                # ---- transpose to feature-major ----
                tp = psA.tile([P, W_], F32, tag="tpose")
                nc.tensor.transpose(tp[:, 0:P], act[:, 0:P], ident)
                nc.tensor.transpose(tp[:, P : 2 * P], act[:, P : 2 * P], ident)
                actT = work.tile([P, W_], F32, tag="actT")
                nc.vector.tensor_copy(actT[:], tp[:])

                if l < 2:
                    phase1_group(l + 1, g,
                                 [actT[:, hf * P : (hf + 1) * P]
                                  for hf in range(n_in_halves[l + 1])])
                else:
                    # poolvec = act3 @ lin_w  -> [128, 1]
                    pv = psA.tile([P, 1], F32, tag="gpsum")
                    for hf in range(W_ // P):
                        nc.tensor.matmul(pv[:], lhsT=actT[:, hf * P : (hf + 1) * P],
                                         rhs=lwsb[hf][:], start=(hf == 0),
                                         stop=(hf == W_ // P - 1))
                    pvsb = work.tile([P, 1], F32, tag="pvsb")
                    nc.vector.tensor_copy(pvsb[:], pv[:])
                    bv = work.tile([P, 1], F32, tag="bv")
                    nc.sync.dma_start(out=bv[:], in_=t_batch[g * P : (g + 1) * P, :])
                    sA = work.tile([P, P], F32, tag="sA")
                    nc.vector.tensor_tensor(out=sA[:], in0=bv[:].to_broadcast([P, P]),
                                            in1=iotaA, op=OP.is_equal)
                    sB = work.tile([P, P], F32, tag="sB")
                    nc.vector.tensor_tensor(out=sB[:], in0=bv[:].to_broadcast([P, P]),
                                            in1=iotaB, op=OP.is_equal)
                    nc.tensor.matmul(poolA[:], lhsT=sA[:], rhs=pvsb[:],
                                     start=(g == 0), stop=(g == NG - 1))
                    nc.tensor.matmul(poolB[:], lhsT=sB[:], rhs=pvsb[:],
                                     start=(g == 0), stop=(g == NG - 1))

        # ---- readout: AllReduce pooled sums, + lin_b ----
        pools = work.tile([P, 2], F32, tag="pools")
        nc.vector.tensor_copy(pools[:, 0:1], poolA[:])
        nc.vector.tensor_copy(pools[:, 1:2], poolB[:])
        ar_in_v = ar_in[:].rearrange("(a p) c -> p (a c)", p=P)
        nc.sync.dma_start(out=ar_in_v, in_=pools[:])
        nc.gpsimd.collective_compute("AllReduce", OP.add, replica_groups=RG,
                                     ins=[ar_in[:].opt()], outs=[ar_out[:].opt()])
        res = work.tile([P, 2], F32, tag="res")
        nc.sync.dma_start(out=res[:], in_=ar_out[:].rearrange("(a p) c -> p (a c)", p=P))
        nc.vector.tensor_scalar_add(res[:], res[:], linb_bc[:, 0:1])
        nc.sync.dma_start(out=t_out[:, :].rearrange("(a p) c -> p (a c)", p=P),
                          in_=res[:])

    nc.compile()
    return nc


# --------------------------------------------------------------------------
# Entry points
# --------------------------------------------------------------------------

def run_gat(inputs: dict, cfg: Cfg = FULL, trace: bool = False):
    in_maps, T = host_prep(cfg, inputs)
    nc = build_nc(cfg, T)
    res = run_bass_kernel_spmd(nc, in_maps, core_ids=list(range(cfg.NCORES)),
                               trace=trace)
    out = np.asarray(res.results[0]["out"], dtype=np.float32).reshape(-1)
    return out[: cfg.NUM_GRAPHS], res


def kernel(**inputs) -> np.ndarray:
    out, _ = run_gat(inputs, FULL, trace=False)
    return out
